# revision 43
# baseline (speedup 1.0000x reference)
"""Trainium2 Bass kernel for nn_DefConv_49005576848085 (topk_masking).

Computes, per batch image (data-parallel over 8 NeuronCores):
  r = dwconv3x3(x, w_r); k = dwconv3x3(x, w_k)            # (576, 96, 96)
  per pixel: softmax over 576 channels of r, top-192 (sorted desc, stable),
  gather k at the top-192 indices, y = [top_r_softmax ; top_k] (384),
  out = w_conv @ y + b_conv                               # (128, 96, 96)

Device pipeline per 128-pixel tile (threshold-compaction top-k):
  PE   : r-conv fp32 + k-conv bf16 as tap-window matmuls -> PSUM
  ACT  : PSUM drains (+ mean/sq accumulators), per-pixel Gaussian-quantile
         threshold theta (Newton rsqrt + 2 exact-count refinements via Sign
         accumulate), shift r by -theta, 16-bit pack/unpack copies
  DVE  : prefix-scan compaction indices, then iterative exact top-8
         extraction x24 over the CW=232-wide compacted positive array
         (max8 / find_index8 / match_replace-to-zero)
  GPSIMD: local_scatter compaction of (r-theta, k) 16-bit halves,
          rank-inversion + 16bit-pair scatter = k-gather
  PE   : transpose sorted arrays, 1x1 conv matmuls (+bias via ACT) -> out

Software pipeline: tile i+1's compaction prep (DVE scan + GPSIMD scatters +
ACT assemble) is injected into the middle of tile i's extraction loop so the
compact array is always ready when the next extraction starts.
"""
import numpy as np
from contextlib import ExitStack

import concourse.bass as bass
import concourse.tile as tile
import concourse.mybir as mybir
from concourse import bacc, library_config
from concourse.bass_utils import run_bass_kernel_spmd

C = 64
M = 576          # C*3*3 conv output channels
OC = 128
TOPK = 192
H = W = 96
NPIX = H * W     # 9216
NB = 8           # batch == cores
NIT = TOPK // 8  # 24 extraction iterations
CW = 232         # compact array width (validated count range [200, 227])
NSPLIT = 12      # trio iteration at which next tile's prep is injected

# threshold constants: target count Ct=224 => z0 = ppf(1-224/576),
# K0 = 1/(576*pdf(z0)); y0 = Newton rsqrt seed for var in [0.115, 0.551];
# second refinement targets Ct2=216 with damping 0.7
CT = 224.0
CT2 = 216.0
DAMP2 = 0.7
Z0 = 0.282216147
K0 = 0.004528583
Y0 = 1.8491254

F32 = mybir.dt.float32
F16 = mybir.dt.float16
BF16 = mybir.dt.bfloat16
I16 = mybir.dt.int16
U16 = mybir.dt.uint16
AF = mybir.ActivationFunctionType
OP = mybir.AluOpType

_CACHE = {}


def build(ntiles=NPIX // 128):
    nc = bacc.Bacc("TRN2", target_bir_lowering=False, debug=False, num_devices=NB)

    x3 = nc.dram_tensor("x3", [C, H, W], F32, kind="ExternalInput").ap()
    x3b = nc.dram_tensor("x3b", [C, H, W], BF16, kind="ExternalInput").ap()
    wdr_d = nc.dram_tensor("wdr", [3, 128, M], F32, kind="ExternalInput").ap()
    wsr_d = nc.dram_tensor("wsr", [3, 64, M], F32, kind="ExternalInput").ap()
    wdk_d = nc.dram_tensor("wdk", [3, 128, M], BF16, kind="ExternalInput").ap()
    wsk_d = nc.dram_tensor("wsk", [3, 64, M], BF16, kind="ExternalInput").ap()
    wfin_d = nc.dram_tensor("wfin", [2 * TOPK, OC], F32, kind="ExternalInput").ap()
    bconv_d = nc.dram_tensor("bconv", [OC, 1], F32, kind="ExternalInput").ap()
    ident_d = nc.dram_tensor("ident", [128, 128], F32, kind="ExternalInput").ap()
    iota1_d = nc.dram_tensor("iota1", [128, TOPK], I16, kind="ExternalInput").ap()
    negone_d = nc.dram_tensor("negone", [128, 1], F32, kind="ExternalInput").ap()
    cb_d = nc.dram_tensor("cb", [128, 4], F32, kind="ExternalInput").ap()
    out_d = nc.dram_tensor("out", [OC, NPIX], F32, kind="ExternalOutput").ap()

    with tile.TileContext(nc) as tc, ExitStack() as ctx:
        nc.gpsimd.load_library(library_config.local_scatter)

        cpool = ctx.enter_context(tc.tile_pool(name="const", bufs=1))
        # x tap-shift planes:
        #  XP partitions 0:64   = X_{-1}[c, q] = x[c, row(q), col(q)-1]  (0 at col 0)
        #  XP partitions 64:128 = X_0  [c, q] = x[c, q]
        #  XQ partitions 0:64   = X_{+1}[c, q] = x[c, row(q), col(q)+1]  (0 at col 95)
        # stored with one zero row before and after (98 rows of 96).
        XP = cpool.tile([128, H + 2, W], F32)
        XQ = cpool.tile([64, H + 2, W], F32)
        XPb = cpool.tile([128, H + 2, W], BF16)
        XQb = cpool.tile([64, H + 2, W], BF16)
        XPf = XP[:].rearrange("p a b -> p (a b)")
        XQf = XQ[:].rearrange("p a b -> p (a b)")
        XPbf = XPb[:].rearrange("p a b -> p (a b)")
        XQbf = XQb[:].rearrange("p a b -> p (a b)")
        for P_, Q_, src in ((XP, XQ, x3), (XPb, XQb, x3b)):
            nc.vector.memset(P_[:, 0, :], 0.0)
            nc.vector.memset(P_[:, H + 1, :], 0.0)
            nc.vector.memset(P_[0:64, 1 : H + 1, 0:1], 0.0)
            nc.vector.memset(Q_[:, 0, :], 0.0)
            nc.vector.memset(Q_[:, H + 1, :], 0.0)
            nc.vector.memset(Q_[0:64, 1 : H + 1, W - 1 : W], 0.0)
            nc.sync.dma_start(P_[64:128, 1 : H + 1, :], src[:, :, :])
            nc.sync.dma_start(P_[0:64, 1 : H + 1, 1:W], src[:, :, 0 : W - 1])
            nc.sync.dma_start(Q_[0:64, 1 : H + 1, 0 : W - 1], src[:, :, 1:W])

        wdr = [cpool.tile([128, M], F32, name=f"wdr{d}", tag=f"wdr{d}") for d in range(3)]
        wsr = [cpool.tile([64, M], F32, name=f"wsr{d}", tag=f"wsr{d}") for d in range(3)]
        wdk = [cpool.tile([128, M], BF16, name=f"wdk{d}", tag=f"wdk{d}") for d in range(3)]
        wsk = [cpool.tile([64, M], BF16, name=f"wsk{d}", tag=f"wsk{d}") for d in range(3)]
        for d in range(3):
            nc.sync.dma_start(wdr[d][:], wdr_d[d])
            nc.sync.dma_start(wsr[d][:], wsr_d[d])
            nc.sync.dma_start(wdk[d][:], wdk_d[d])
            nc.sync.dma_start(wsk[d][:], wsk_d[d])
        wf1 = cpool.tile([128, OC], F32)
        wf2 = cpool.tile([64, OC], F32)
        wf3 = cpool.tile([128, OC], F32)
        wf4 = cpool.tile([64, OC], F32)
        nc.sync.dma_start(wf1[:], wfin_d[0:128])
        nc.sync.dma_start(wf2[:], wfin_d[128:192])
        nc.sync.dma_start(wf3[:], wfin_d[192:320])
        nc.sync.dma_start(wf4[:], wfin_d[320:384])
        ident = cpool.tile([128, 128], F32)
        nc.sync.dma_start(ident[:], ident_d[:])
        iota1 = cpool.tile([128, TOPK], I16)
        nc.sync.dma_start(iota1[:], iota1_d[:])
        bconv = cpool.tile([OC, 1], F32)
        nc.sync.dma_start(bconv[:], bconv_d[:])
        negone = cpool.tile([128, 1], F32)
        nc.sync.dma_start(negone[:], negone_d[:])
        cb = cpool.tile([128, 4], F32)  # cols: [1.5*Y0, 1.5, 576-2*Ct, 576-2*Ct2]
        nc.sync.dma_start(cb[:], cb_d[:])
        zeros = cpool.tile([128, M], F32)
        nc.vector.memset(zeros[:], 0.0)

        pool = ctx.enter_context(tc.tile_pool(name="work", bufs=2))
        pool3 = ctx.enter_context(tc.tile_pool(name="work3", bufs=3))
        psum = ctx.enter_context(tc.tile_pool(name="psum", bufs=1, space="PSUM"))

        def tiny(tag):
            return pool.tile([128, 1], F32, tag=tag, name=tag)

        def emit_fa(it):
            """Convs, drains, theta estimation, shifted r for tile `it`.
            No DVE work (so it can be emitted ahead of the previous trio)."""
            p0 = 128 * it
            pr1 = psum.tile([128, 288], F32, tag="pr1")
            pr2 = psum.tile([128, 288], F32, tag="pr2")
            pk1 = psum.tile([128, 288], F32, tag="pk1")
            pk2 = psum.tile([128, 288], F32, tag="pk2")
            for d in range(3):  # dy = d - 1; taps (dy,-1),(dy,0) dual; (dy,+1) single
                w0 = 96 * d + p0
                lhd = XPf[:, w0 : w0 + 128]
                lhs = XQf[0:64, w0 : w0 + 128]
                lhdb = XPbf[:, w0 : w0 + 128]
                lhsb = XQbf[0:64, w0 : w0 + 128]
                st = d == 0
                sp = d == 2
                nc.tensor.matmul(pr1[:], lhd, wdr[d][:, 0:288], start=st, stop=False)
                nc.tensor.matmul(pr2[:], lhd, wdr[d][:, 288:M], start=st, stop=False)
                nc.tensor.matmul(pk1[:], lhdb, wdk[d][:, 0:288], start=st, stop=False)
                nc.tensor.matmul(pk2[:], lhdb, wdk[d][:, 288:M], start=st, stop=False)
                nc.tensor.matmul(pr1[:], lhs, wsr[d][:, 0:288], start=False, stop=sp)
                nc.tensor.matmul(pr2[:], lhs, wsr[d][:, 288:M], start=False, stop=sp)
                nc.tensor.matmul(pk1[:], lhsb, wsk[d][:, 0:288], start=False, stop=sp)
                nc.tensor.matmul(pk2[:], lhsb, wsk[d][:, 288:M], start=False, stop=sp)

            # drains + stats; rs doubles as scratch for Square/Sign passes
            r = pool.tile([128, M], F32, tag="r")
            kv = pool.tile([128, M], F32, tag="kv")
            rs = pool3.tile([128, M], F32, tag="rs")
            s1 = tiny("s1")
            s2 = tiny("s2")
            q1 = tiny("q1")
            q2 = tiny("q2")
            nc.scalar.activation(r[:, 0:288], pr1[:], AF.Identity, accum_out=s1[:])
            nc.scalar.activation(r[:, 288:M], pr2[:], AF.Identity, accum_out=s2[:])
            nc.scalar.activation(kv[:, 0:288], pk1[:], AF.Identity)
            nc.scalar.activation(kv[:, 288:M], pk2[:], AF.Identity)
            nc.scalar.activation(rs[:, 0:288], pr1[:], AF.Square, accum_out=q1[:])
            nc.scalar.activation(rs[:, 288:M], pr2[:], AF.Square, accum_out=q2[:])

            # theta estimate (tiny ACT chain; out = func(in*scale + bias))
            S = tiny("S")
            q = tiny("q")
            nc.scalar.activation(S[:], s1[:], AF.Identity, bias=s2[:])
            nc.scalar.activation(q[:], q1[:], AF.Identity, bias=q2[:])
            negmu = tiny("negmu")
            msq = tiny("msq")
            mu2 = tiny("mu2")
            var = tiny("var")
            nc.scalar.activation(negmu[:], S[:], AF.Identity, scale=-1.0 / M)
            nc.scalar.activation(msq[:], q[:], AF.Identity, scale=1.0 / M)
            nc.scalar.activation(mu2[:], negmu[:], AF.Square)
            nc.scalar.activation(var[:], mu2[:], AF.Identity, scale=-1.0, bias=msq[:])
            # sg ~= sqrt(var) via 1 Newton rsqrt step from seed Y0 (the two
            # exact-count refinements absorb the remaining ~10% sg error)
            y1 = tiny("y1")
            sg = tiny("sg")
            nc.scalar.activation(y1[:], var[:], AF.Identity,
                                 scale=-0.5 * Y0 ** 3, bias=cb[:, 0:1])
            nc.scalar.activation(sg[:], var[:], AF.Identity, scale=y1[:])
            negth0 = tiny("negth0")
            nc.scalar.activation(negth0[:], sg[:], AF.Identity, scale=-Z0,
                                 bias=negmu[:])
            # refinement 1: exact count at theta0 via Sign accumulate
            # (ssum = 2*count - 576):
            #   negth1 = negth0 + (ssum + (576-2*Ct)) * (-K0/2*sg)
            ssum = tiny("ssum")
            nc.scalar.activation(rs[:], r[:], AF.Sign, bias=negth0[:],
                                 accum_out=ssum[:])
            uK2 = tiny("uK2")
            sh = tiny("sh")
            d2 = tiny("d2")
            negth1 = tiny("negth1")
            nc.scalar.activation(uK2[:], sg[:], AF.Identity, scale=-K0 / 2.0)
            nc.scalar.activation(sh[:], ssum[:], AF.Identity, bias=cb[:, 2:3])
            nc.scalar.activation(d2[:], sh[:], AF.Identity, scale=uK2[:])
            nc.scalar.activation(negth1[:], d2[:], AF.Identity, bias=negth0[:])
            # refinement 2 (damped) targeting Ct2
            ssum2 = tiny("ssum2")
            nc.scalar.activation(rs[:], r[:], AF.Sign, bias=negth1[:],
                                 accum_out=ssum2[:])
            uK3 = tiny("uK3")
            sh2 = tiny("sh2")
            d3 = tiny("d3")
            negth2 = tiny("negth2")
            nc.scalar.activation(uK3[:], sg[:], AF.Identity,
                                 scale=-K0 * DAMP2 / 2.0)
            nc.scalar.activation(sh2[:], ssum2[:], AF.Identity, bias=cb[:, 3:4])
            nc.scalar.activation(d3[:], sh2[:], AF.Identity, scale=uK3[:])
            nc.scalar.activation(negth2[:], d3[:], AF.Identity, bias=negth1[:])
            # shifted r (>= 0 exactly on the kept elements)
            nc.scalar.activation(rs[:], r[:], AF.Identity, bias=negth2[:])
            return dict(p0=p0, r=r, kv=kv, rs=rs)

        def emit_prep(h):
            """Compaction for tile of `h`: DVE scan + scatters + assemble.
            The 16-bit splits are emitted first so ACT runs them while DVE
            does the scan; si16 stays on DVE to avoid an extra ACT hop in
            front of the scatters. Returns handles for the trio + post."""
            rs, kv = h["rs"], h["kv"]
            rslo = pool.tile([128, M], U16, tag="rslo")
            rshi = pool.tile([128, M], U16, tag="rshi")
            rsu = rs[:].bitcast(U16)
            nc.scalar.activation(rslo[:], rsu[:, 0 : 2 * M : 2], AF.Copy)
            nc.scalar.activation(rshi[:], rsu[:, 1 : 2 * M : 2], AF.Copy)
            ind = pool.tile([128, M], BF16, tag="ind")
            nc.vector.tensor_scalar(ind[:], rs[:], 0.0, None, OP.is_ge)
            pc = pool.tile([128, M], F32, tag="r")  # r is dead; reuse as pc
            nc.vector.tensor_tensor_scan(pc[:], ind[:], zeros[:], 0.0, OP.add, OP.add)
            siF = pool.tile([128, M], F16, tag="siF")
            nc.vector.scalar_tensor_tensor(siF[:], rs[:], 0.0, pc[:], OP.is_ge, OP.mult)
            si16 = pool.tile([128, M], I16, tag="si16")
            nc.vector.tensor_scalar(si16[:], siF[:], -1.0, None, OP.add)
            aclo = pool.tile([128, CW], U16, tag="aclo")
            achi = pool.tile([128, CW], U16, tag="achi")
            nc.gpsimd.local_scatter(aclo[:], rslo[:], si16[:],
                                    channels=128, num_elems=CW, num_idxs=M)
            nc.gpsimd.local_scatter(achi[:], rshi[:], si16[:],
                                    channels=128, num_elems=CW, num_idxs=M)
            ac = pool.tile([128, CW], F32, tag="ac")
            acu = ac[:].bitcast(U16)
            nc.scalar.activation(acu[:, 0 : 2 * CW : 2], aclo[:], AF.Copy)
            nc.scalar.activation(acu[:, 1 : 2 * CW : 2], achi[:], AF.Copy)

            # k compaction (consumed only by post; scatters overlap the trio)
            klo = pool.tile([128, M], U16, tag="klo")
            khi = pool.tile([128, M], U16, tag="khi")
            kvu = kv[:].bitcast(U16)
            nc.scalar.activation(klo[:], kvu[:, 0 : 2 * M : 2], AF.Copy)
            nc.scalar.activation(khi[:], kvu[:, 1 : 2 * M : 2], AF.Copy)
            cklo = pool.tile([128, CW], U16, tag="cklo")
            ckhi = pool.tile([128, CW], U16, tag="ckhi")
            nc.gpsimd.local_scatter(cklo[:], klo[:], si16[:],
                                    channels=128, num_elems=CW, num_idxs=M)
            nc.gpsimd.local_scatter(ckhi[:], khi[:], si16[:],
                                    channels=128, num_elems=CW, num_idxs=M)
            h.update(ac=ac, cklo=cklo, ckhi=ckhi)
            return h

        def emit_trio(h, mid=None):
            """Top-192 extraction for tile of `h`; `mid` emits the next
            tile's prep after iteration NSPLIT so its scatters overlap."""
            ac = h["ac"]
            maxs = pool3.tile([128, TOPK], F32, tag="maxs")
            idxu = pool.tile([128, TOPK], U16, tag="idxu")
            for t in range(NIT):
                if t == NSPLIT and mid is not None:
                    mid()
                m8 = maxs[:, 8 * t : 8 * t + 8]
                nc.vector.max(out=m8, in_=ac[:])
                nc.vector.max_index(out=idxu[:, 8 * t : 8 * t + 8], in_max=m8,
                                    in_values=ac[:])
                if t < NIT - 1:
                    # zero out the extracted 8 (all values >= 0 in the shifted
                    # domain, so 0 acts as -inf)
                    nc.vector.match_replace(out=ac[:], in_to_replace=m8,
                                            in_values=ac[:], imm_value=0.0)
            h.update(maxs=maxs, idxu=idxu)
            return h

        def emit_post_k(h):
            """Rank inversion + k gather — emitted right after the tile's own
            trio so the GPSIMD round-trips complete before they are needed."""
            idxu, cklo, ckhi = h["idxu"], h["cklo"], h["ckhi"]
            rankp1 = pool.tile([128, CW], I16, tag="rankp1")
            nc.gpsimd.local_scatter(
                rankp1[:], iota1[:], idxu[:].bitcast(I16),
                channels=128, num_elems=CW, num_idxs=TOPK)
            rankm1 = pool.tile([128, CW], I16, tag="rankm1")
            nc.scalar.activation(rankm1[:], rankp1[:], AF.Identity, bias=negone[:])
            kglo = pool.tile([128, TOPK], U16, tag="kglo")
            kghi = pool.tile([128, TOPK], U16, tag="kghi")
            nc.gpsimd.local_scatter(kglo[:], cklo[:], rankm1[:],
                                    channels=128, num_elems=TOPK, num_idxs=CW)
            nc.gpsimd.local_scatter(kghi[:], ckhi[:], rankm1[:],
                                    channels=128, num_elems=TOPK, num_idxs=CW)
            tkk = pool.tile([128, TOPK], F32, tag="tkk")
            tkku = tkk[:].bitcast(U16)
            nc.scalar.activation(tkku[:, 0 : 2 * TOPK : 2], kglo[:], AF.Copy)
            nc.scalar.activation(tkku[:, 1 : 2 * TOPK : 2], kghi[:], AF.Copy)
            h.update(tkk=tkk)
            return h

        def emit_post_soft(h):
            """Softmax pieces, transposes, final conv (deferred one tile)."""
            p0, rs, maxs, tkk = h["p0"], h["rs"], h["maxs"], h["tkk"]
            negm = tiny("negm")
            nc.scalar.activation(negm[:], maxs[:, 0:1], AF.Identity, scale=-1.0)
            junk2 = pool.tile([128, M], F32, tag="r", name="junk2")  # scratch
            zsum = tiny("zsum")
            nc.scalar.activation(junk2[:], rs[:], AF.Exp, bias=negm[:],
                                 accum_out=zsum[:])
            rz = tiny("rz")
            nc.vector.reciprocal(rz[:], zsum[:])
            esort = pool.tile([128, TOPK], F32, tag="esort")
            nc.scalar.activation(esort[:], maxs[:], AF.Exp, bias=negm[:])
            nc.scalar.activation(esort[:], esort[:], AF.Copy, bias=0.0, scale=rz[:])

            # y^T via PE transpose
            yt1 = pool.tile([128, 128], F32, tag="yt1")
            yt2 = pool.tile([64, 128], F32, tag="yt2")
            yt3 = pool.tile([128, 128], F32, tag="yt3")
            yt4 = pool.tile([64, 128], F32, tag="yt4")
            for src, dst, width in ((esort[:, 0:128], yt1, 128),
                                    (esort[:, 128:192], yt2, 64),
                                    (tkk[:, 0:128], yt3, 128),
                                    (tkk[:, 128:192], yt4, 64)):
                tps = psum.tile([width, 128], F32, name="tps", tag="tps")
                nc.tensor.transpose(tps[:], src, ident[:])
                nc.scalar.activation(dst[:], tps[:], AF.Identity)

            # final 1x1 conv
            outp = psum.tile([OC, 128], F32, tag="outp")
            nc.tensor.matmul(outp[:], wf1[:], yt1[:], start=True, stop=False)
            nc.tensor.matmul(outp[:], wf2[:], yt2[:], start=False, stop=False)
            nc.tensor.matmul(outp[:], wf3[:], yt3[:], start=False, stop=False)
            nc.tensor.matmul(outp[:], wf4[:], yt4[:], start=False, stop=True)
            outsb = pool.tile([OC, 128], F32, tag="outsb")
            nc.scalar.activation(outsb[:], outp[:], AF.Identity, bias=bconv[:])
            nc.sync.dma_start(out_d[:, p0 : p0 + 128], outsb[:])

        # software pipeline:
        #   fa(0); prep(0)
        #   for i: fa(i+1); trio(i){mid: prep(i+1)}; post_k(i); post_soft(i-1)
        #   post_soft(ntiles-1)
        hs = [None] * ntiles
        hs[0] = emit_prep(emit_fa(0))
        for i in range(ntiles):
            if i + 1 < ntiles:
                hs[i + 1] = emit_fa(i + 1)
                mid = (lambda j=i + 1: emit_prep(hs[j]))
            else:
                mid = None
            emit_trio(hs[i], mid=mid)
            emit_post_k(hs[i])
            if i >= 1:
                emit_post_soft(hs[i - 1])
        emit_post_soft(hs[ntiles - 1])

    nc.compile()
    return nc


def host_inputs(x, w_r, w_k, w_conv, b_conv):
    """Build the per-core in_maps (host side: only slicing/layout, no math)."""
    import ml_dtypes
    wr = w_r[:, 0]  # (576, 3, 3)
    wk = w_k[:, 0]
    g = np.arange(M) // 9  # group (input channel) of each output channel

    def dual(wv, dy):  # (128, 576): rows 0:64 tap (dy,-1), rows 64:128 tap (dy,0)
        m = np.zeros((128, M), np.float32)
        m[g, np.arange(M)] = wv[:, dy, 0]
        m[64 + g, np.arange(M)] = wv[:, dy, 1]
        return m

    def single(wv, dy):  # (64, 576): tap (dy,+1)
        m = np.zeros((64, M), np.float32)
        m[g, np.arange(M)] = wv[:, dy, 2]
        return m

    wdr = np.stack([dual(wr, d) for d in range(3)])
    wsr = np.stack([single(wr, d) for d in range(3)])
    wdk = np.stack([dual(wk, d) for d in range(3)]).astype(ml_dtypes.bfloat16)
    wsk = np.stack([single(wk, d) for d in range(3)]).astype(ml_dtypes.bfloat16)
    wfin = np.ascontiguousarray(w_conv[:, :, 0, 0].T.astype(np.float32))  # (384, 128)
    bc = np.ascontiguousarray(b_conv.astype(np.float32).reshape(OC, 1))
    ident = np.eye(128, dtype=np.float32)
    iota1 = np.tile(np.arange(1, TOPK + 1, dtype=np.int16), (128, 1))
    negone = np.full((128, 1), -1.0, np.float32)
    cb = np.tile(np.array([1.5 * Y0, 1.5, M - 2 * CT, M - 2 * CT2], np.float32),
                 (128, 1))
    consts = dict(wdr=wdr, wsr=wsr, wdk=wdk, wsk=wsk, wfin=wfin, bconv=bc,
                  ident=ident, iota1=iota1, negone=negone, cb=cb)
    return [dict(x3=np.ascontiguousarray(x[b].astype(np.float32)),
                 x3b=np.ascontiguousarray(x[b].astype(ml_dtypes.bfloat16)),
                 **consts)
            for b in range(NB)]


def kernel(x, w_r, w_k, w_conv, b_conv):
    if "nc" not in _CACHE:
        _CACHE["nc"] = build()
    nc = _CACHE["nc"]
    in_maps = host_inputs(np.asarray(x), np.asarray(w_r), np.asarray(w_k),
                          np.asarray(w_conv), np.asarray(b_conv))
    res = run_bass_kernel_spmd(nc, in_maps, list(range(NB)))
    out = np.stack([res.results[b]["out"] for b in range(NB)], axis=0)
    return out.reshape(NB, OC, H, W).astype(np.float32)


# revision 45
# speedup vs baseline: 1.1521x; 1.1521x over previous
"""Trainium2 Bass kernel for nn_DefConv_49005576848085 (topk_masking).

Computes, per batch image (data-parallel over 8 NeuronCores):
  r = dwconv3x3(x, w_r); k = dwconv3x3(x, w_k)            # (576, 96, 96)
  per pixel: softmax over 576 channels of r, top-192 (sorted desc, stable),
  gather k at the top-192 indices, y = [top_r_softmax ; top_k] (384),
  out = w_conv @ y + b_conv                               # (128, 96, 96)

Device pipeline per 128-pixel tile (threshold-compaction top-k):
  PE   : r-conv fp32 + k-conv bf16 as tap-window matmuls -> PSUM
  ACT  : PSUM drains (+ mean/sq accumulators), per-pixel Gaussian-quantile
         threshold theta (Newton rsqrt + 2 exact-count refinements via Sign
         accumulate), shift r by -theta, 16-bit pack/unpack copies
  DVE  : prefix-scan compaction indices, then iterative exact top-8
         extraction x24 over the CW=232-wide compacted positive array
         (max8 / find_index8 / match_replace-to-zero)
  GPSIMD: local_scatter compaction of (r-theta, k) 16-bit halves,
          rank-inversion + 16bit-pair scatter = k-gather
  PE   : transpose sorted arrays, 1x1 conv matmuls (+bias via ACT) -> out

Software pipeline: tile i+1's compaction prep (DVE scan + GPSIMD scatters +
ACT assemble) is injected into the middle of tile i's extraction loop so the
compact array is always ready when the next extraction starts.
"""
import numpy as np
from contextlib import ExitStack

import concourse.bass as bass
import concourse.tile as tile
import concourse.mybir as mybir
from concourse import bacc, library_config
from concourse.bass_utils import run_bass_kernel_spmd

C = 64
M = 576          # C*3*3 conv output channels
OC = 128
TOPK = 192
H = W = 96
NPIX = H * W     # 9216
NB = 8           # batch == cores
NIT = TOPK // 8  # 24 extraction iterations
CW = 232         # compact array width (validated count range [200, 227])
NSPLIT = 16      # trio iteration at which next tile's prep is injected

# threshold constants: target count Ct=224 => z0 = ppf(1-224/576),
# K0 = 1/(576*pdf(z0)); y0 = Newton rsqrt seed for var in [0.115, 0.551];
# second refinement targets Ct2=216 with damping 0.7
CT = 224.0
CT2 = 216.0
DAMP2 = 0.7
Z0 = 0.282216147
K0 = 0.004528583
Y0 = 1.8491254

F32 = mybir.dt.float32
F16 = mybir.dt.float16
BF16 = mybir.dt.bfloat16
I16 = mybir.dt.int16
U16 = mybir.dt.uint16
AF = mybir.ActivationFunctionType
OP = mybir.AluOpType

_CACHE = {}


def build(ntiles=NPIX // 128):
    nc = bacc.Bacc("TRN2", target_bir_lowering=False, debug=False, num_devices=NB)

    x3 = nc.dram_tensor("x3", [C, H, W], F32, kind="ExternalInput").ap()
    x3b = nc.dram_tensor("x3b", [C, H, W], BF16, kind="ExternalInput").ap()
    wdr_d = nc.dram_tensor("wdr", [3, 128, M], F32, kind="ExternalInput").ap()
    wsr_d = nc.dram_tensor("wsr", [3, 64, M], F32, kind="ExternalInput").ap()
    wdk_d = nc.dram_tensor("wdk", [3, 128, M], BF16, kind="ExternalInput").ap()
    wsk_d = nc.dram_tensor("wsk", [3, 64, M], BF16, kind="ExternalInput").ap()
    wfin_d = nc.dram_tensor("wfin", [2 * TOPK, OC], F32, kind="ExternalInput").ap()
    bconv_d = nc.dram_tensor("bconv", [OC, 1], F32, kind="ExternalInput").ap()
    ident_d = nc.dram_tensor("ident", [128, 128], F32, kind="ExternalInput").ap()
    iota1_d = nc.dram_tensor("iota1", [128, TOPK], I16, kind="ExternalInput").ap()
    negone_d = nc.dram_tensor("negone", [128, 1], F32, kind="ExternalInput").ap()
    cb_d = nc.dram_tensor("cb", [128, 4], F32, kind="ExternalInput").ap()
    out_d = nc.dram_tensor("out", [OC, NPIX], F32, kind="ExternalOutput").ap()

    with tile.TileContext(nc) as tc, ExitStack() as ctx:
        nc.gpsimd.load_library(library_config.local_scatter)

        cpool = ctx.enter_context(tc.tile_pool(name="const", bufs=1))
        # x tap-shift planes:
        #  XP partitions 0:64   = X_{-1}[c, q] = x[c, row(q), col(q)-1]  (0 at col 0)
        #  XP partitions 64:128 = X_0  [c, q] = x[c, q]
        #  XQ partitions 0:64   = X_{+1}[c, q] = x[c, row(q), col(q)+1]  (0 at col 95)
        # stored with one zero row before and after (98 rows of 96).
        XP = cpool.tile([128, H + 2, W], F32)
        XQ = cpool.tile([64, H + 2, W], F32)
        XPb = cpool.tile([128, H + 2, W], BF16)
        XQb = cpool.tile([64, H + 2, W], BF16)
        XPf = XP[:].rearrange("p a b -> p (a b)")
        XQf = XQ[:].rearrange("p a b -> p (a b)")
        XPbf = XPb[:].rearrange("p a b -> p (a b)")
        XQbf = XQb[:].rearrange("p a b -> p (a b)")
        for P_, Q_, src in ((XP, XQ, x3), (XPb, XQb, x3b)):
            nc.vector.memset(P_[:, 0, :], 0.0)
            nc.vector.memset(P_[:, H + 1, :], 0.0)
            nc.vector.memset(P_[0:64, 1 : H + 1, 0:1], 0.0)
            nc.vector.memset(Q_[:, 0, :], 0.0)
            nc.vector.memset(Q_[:, H + 1, :], 0.0)
            nc.vector.memset(Q_[0:64, 1 : H + 1, W - 1 : W], 0.0)
            nc.sync.dma_start(P_[64:128, 1 : H + 1, :], src[:, :, :])
            nc.sync.dma_start(P_[0:64, 1 : H + 1, 1:W], src[:, :, 0 : W - 1])
            nc.sync.dma_start(Q_[0:64, 1 : H + 1, 0 : W - 1], src[:, :, 1:W])

        wdr = [cpool.tile([128, M], F32, name=f"wdr{d}", tag=f"wdr{d}") for d in range(3)]
        wsr = [cpool.tile([64, M], F32, name=f"wsr{d}", tag=f"wsr{d}") for d in range(3)]
        wdk = [cpool.tile([128, M], BF16, name=f"wdk{d}", tag=f"wdk{d}") for d in range(3)]
        wsk = [cpool.tile([64, M], BF16, name=f"wsk{d}", tag=f"wsk{d}") for d in range(3)]
        for d in range(3):
            nc.sync.dma_start(wdr[d][:], wdr_d[d])
            nc.sync.dma_start(wsr[d][:], wsr_d[d])
            nc.sync.dma_start(wdk[d][:], wdk_d[d])
            nc.sync.dma_start(wsk[d][:], wsk_d[d])
        wf1 = cpool.tile([128, OC], F32)
        wf2 = cpool.tile([64, OC], F32)
        wf3 = cpool.tile([128, OC], F32)
        wf4 = cpool.tile([64, OC], F32)
        nc.sync.dma_start(wf1[:], wfin_d[0:128])
        nc.sync.dma_start(wf2[:], wfin_d[128:192])
        nc.sync.dma_start(wf3[:], wfin_d[192:320])
        nc.sync.dma_start(wf4[:], wfin_d[320:384])
        ident = cpool.tile([128, 128], F32)
        nc.sync.dma_start(ident[:], ident_d[:])
        iota1 = cpool.tile([128, TOPK], I16)
        nc.sync.dma_start(iota1[:], iota1_d[:])
        bconv = cpool.tile([OC, 1], F32)
        nc.sync.dma_start(bconv[:], bconv_d[:])
        negone = cpool.tile([128, 1], F32)
        nc.sync.dma_start(negone[:], negone_d[:])
        cb = cpool.tile([128, 4], F32)  # cols: [1.5*Y0, 1.5, 576-2*Ct, 576-2*Ct2]
        nc.sync.dma_start(cb[:], cb_d[:])
        zeros = cpool.tile([128, M], F32)
        nc.vector.memset(zeros[:], 0.0)

        pool = ctx.enter_context(tc.tile_pool(name="work", bufs=2))
        pool3 = ctx.enter_context(tc.tile_pool(name="work3", bufs=3))
        psum = ctx.enter_context(tc.tile_pool(name="psum", bufs=1, space="PSUM"))

        def tiny(tag):
            return pool.tile([128, 1], F32, tag=tag, name=tag)

        def emit_fa(it):
            """Convs, drains, theta estimation, shifted r for tile `it`.
            No DVE work (so it can be emitted ahead of the previous trio)."""
            p0 = 128 * it
            pr1 = psum.tile([128, 288], F32, tag="pr1")
            pr2 = psum.tile([128, 288], F32, tag="pr2")
            pk1 = psum.tile([128, 288], F32, tag="pk1")
            pk2 = psum.tile([128, 288], F32, tag="pk2")
            for d in range(3):  # dy = d - 1; taps (dy,-1),(dy,0) dual; (dy,+1) single
                w0 = 96 * d + p0
                lhd = XPf[:, w0 : w0 + 128]
                lhs = XQf[0:64, w0 : w0 + 128]
                lhdb = XPbf[:, w0 : w0 + 128]
                lhsb = XQbf[0:64, w0 : w0 + 128]
                st = d == 0
                sp = d == 2
                nc.tensor.matmul(pr1[:], lhd, wdr[d][:, 0:288], start=st, stop=False)
                nc.tensor.matmul(pr2[:], lhd, wdr[d][:, 288:M], start=st, stop=False)
                nc.tensor.matmul(pk1[:], lhdb, wdk[d][:, 0:288], start=st, stop=False)
                nc.tensor.matmul(pk2[:], lhdb, wdk[d][:, 288:M], start=st, stop=False)
                nc.tensor.matmul(pr1[:], lhs, wsr[d][:, 0:288], start=False, stop=sp)
                nc.tensor.matmul(pr2[:], lhs, wsr[d][:, 288:M], start=False, stop=sp)
                nc.tensor.matmul(pk1[:], lhsb, wsk[d][:, 0:288], start=False, stop=sp)
                nc.tensor.matmul(pk2[:], lhsb, wsk[d][:, 288:M], start=False, stop=sp)

            # drains + stats; rs doubles as scratch for Square/Sign passes
            r = pool.tile([128, M], F32, tag="r")
            kv = pool.tile([128, M], F32, tag="kv")
            rs = pool3.tile([128, M], F32, tag="rs")
            s1 = tiny("s1")
            s2 = tiny("s2")
            q1 = tiny("q1")
            q2 = tiny("q2")
            nc.scalar.activation(r[:, 0:288], pr1[:], AF.Identity, accum_out=s1[:])
            nc.scalar.activation(r[:, 288:M], pr2[:], AF.Identity, accum_out=s2[:])
            nc.scalar.activation(kv[:, 0:288], pk1[:], AF.Identity)
            nc.scalar.activation(kv[:, 288:M], pk2[:], AF.Identity)
            nc.scalar.activation(rs[:, 0:288], pr1[:], AF.Square, accum_out=q1[:])
            nc.scalar.activation(rs[:, 288:M], pr2[:], AF.Square, accum_out=q2[:])

            # theta estimate (tiny ACT chain; out = func(in*scale + bias))
            S = tiny("S")
            q = tiny("q")
            nc.scalar.activation(S[:], s1[:], AF.Identity, bias=s2[:])
            nc.scalar.activation(q[:], q1[:], AF.Identity, bias=q2[:])
            negmu = tiny("negmu")
            msq = tiny("msq")
            mu2 = tiny("mu2")
            var = tiny("var")
            nc.scalar.activation(negmu[:], S[:], AF.Identity, scale=-1.0 / M)
            nc.scalar.activation(msq[:], q[:], AF.Identity, scale=1.0 / M)
            nc.scalar.activation(mu2[:], negmu[:], AF.Square)
            nc.scalar.activation(var[:], mu2[:], AF.Identity, scale=-1.0, bias=msq[:])
            # sg ~= sqrt(var) via 1 Newton rsqrt step from seed Y0 (the two
            # exact-count refinements absorb the remaining ~10% sg error)
            y1 = tiny("y1")
            sg = tiny("sg")
            nc.scalar.activation(y1[:], var[:], AF.Identity,
                                 scale=-0.5 * Y0 ** 3, bias=cb[:, 0:1])
            nc.scalar.activation(sg[:], var[:], AF.Identity, scale=y1[:])
            negth0 = tiny("negth0")
            nc.scalar.activation(negth0[:], sg[:], AF.Identity, scale=-Z0,
                                 bias=negmu[:])
            # refinement 1: exact count at theta0 via Sign accumulate
            # (ssum = 2*count - 576):
            #   negth1 = negth0 + (ssum + (576-2*Ct)) * (-K0/2*sg)
            ssum = tiny("ssum")
            nc.scalar.activation(rs[:], r[:], AF.Sign, bias=negth0[:],
                                 accum_out=ssum[:])
            uK2 = tiny("uK2")
            sh = tiny("sh")
            d2 = tiny("d2")
            negth1 = tiny("negth1")
            nc.scalar.activation(uK2[:], sg[:], AF.Identity, scale=-K0 / 2.0)
            nc.scalar.activation(sh[:], ssum[:], AF.Identity, bias=cb[:, 2:3])
            nc.scalar.activation(d2[:], sh[:], AF.Identity, scale=uK2[:])
            nc.scalar.activation(negth1[:], d2[:], AF.Identity, bias=negth0[:])
            # refinement 2 (damped) targeting Ct2
            ssum2 = tiny("ssum2")
            nc.scalar.activation(rs[:], r[:], AF.Sign, bias=negth1[:],
                                 accum_out=ssum2[:])
            uK3 = tiny("uK3")
            sh2 = tiny("sh2")
            d3 = tiny("d3")
            negth2 = tiny("negth2")
            nc.scalar.activation(uK3[:], sg[:], AF.Identity,
                                 scale=-K0 * DAMP2 / 2.0)
            nc.scalar.activation(sh2[:], ssum2[:], AF.Identity, bias=cb[:, 3:4])
            nc.scalar.activation(d3[:], sh2[:], AF.Identity, scale=uK3[:])
            nc.scalar.activation(negth2[:], d3[:], AF.Identity, bias=negth1[:])
            # shifted r (>= 0 exactly on the kept elements)
            nc.scalar.activation(rs[:], r[:], AF.Identity, bias=negth2[:])
            return dict(p0=p0, r=r, kv=kv, rs=rs)

        def emit_prep(h):
            """Compaction for tile of `h`: DVE scan + scatters + assemble.
            Returns handles for the trio + post."""
            rs, kv = h["rs"], h["kv"]
            ind = pool.tile([128, M], BF16, tag="ind")
            nc.vector.tensor_scalar(ind[:], rs[:], 0.0, None, OP.is_ge)
            pc = pool.tile([128, M], F32, tag="r")  # r is dead; reuse as pc
            nc.vector.tensor_tensor_scan(pc[:], ind[:], zeros[:], 0.0, OP.add, OP.add)
            siF = pool.tile([128, M], F16, tag="siF")
            nc.vector.scalar_tensor_tensor(siF[:], rs[:], 0.0, pc[:], OP.is_ge, OP.mult)
            si16 = pool.tile([128, M], I16, tag="si16")
            nc.scalar.activation(si16[:], siF[:], AF.Identity, bias=negone[:])

            rslo = pool.tile([128, M], U16, tag="rslo")
            rshi = pool.tile([128, M], U16, tag="rshi")
            rsu = rs[:].bitcast(U16)
            nc.scalar.activation(rslo[:], rsu[:, 0 : 2 * M : 2], AF.Copy)
            nc.scalar.activation(rshi[:], rsu[:, 1 : 2 * M : 2], AF.Copy)
            aclo = pool.tile([128, CW], U16, tag="aclo")
            achi = pool.tile([128, CW], U16, tag="achi")
            nc.gpsimd.local_scatter(aclo[:], rslo[:], si16[:],
                                    channels=128, num_elems=CW, num_idxs=M)
            nc.gpsimd.local_scatter(achi[:], rshi[:], si16[:],
                                    channels=128, num_elems=CW, num_idxs=M)
            ac = pool.tile([128, CW], F32, tag="ac")
            acu = ac[:].bitcast(U16)
            nc.scalar.activation(acu[:, 0 : 2 * CW : 2], aclo[:], AF.Copy)
            nc.scalar.activation(acu[:, 1 : 2 * CW : 2], achi[:], AF.Copy)

            # k compaction (consumed only by post; scatters overlap the trio)
            klo = pool.tile([128, M], U16, tag="klo")
            khi = pool.tile([128, M], U16, tag="khi")
            kvu = kv[:].bitcast(U16)
            nc.scalar.activation(klo[:], kvu[:, 0 : 2 * M : 2], AF.Copy)
            nc.scalar.activation(khi[:], kvu[:, 1 : 2 * M : 2], AF.Copy)
            cklo = pool.tile([128, CW], U16, tag="cklo")
            ckhi = pool.tile([128, CW], U16, tag="ckhi")
            nc.gpsimd.local_scatter(cklo[:], klo[:], si16[:],
                                    channels=128, num_elems=CW, num_idxs=M)
            nc.gpsimd.local_scatter(ckhi[:], khi[:], si16[:],
                                    channels=128, num_elems=CW, num_idxs=M)
            h.update(ac=ac, cklo=cklo, ckhi=ckhi)
            return h

        def emit_trio(h, mid=None):
            """Top-192 extraction for tile of `h`; `mid` emits the next
            tile's prep after iteration NSPLIT so its scatters overlap."""
            ac = h["ac"]
            maxs = pool3.tile([128, TOPK], F32, tag="maxs")
            idxu = pool.tile([128, TOPK], U16, tag="idxu")
            for t in range(NIT):
                if t == NSPLIT and mid is not None:
                    mid()
                m8 = maxs[:, 8 * t : 8 * t + 8]
                nc.vector.max(out=m8, in_=ac[:])
                nc.vector.max_index(out=idxu[:, 8 * t : 8 * t + 8], in_max=m8,
                                    in_values=ac[:])
                if t < NIT - 1:
                    # zero out the extracted 8 (all values >= 0 in the shifted
                    # domain, so 0 acts as -inf)
                    nc.vector.match_replace(out=ac[:], in_to_replace=m8,
                                            in_values=ac[:], imm_value=0.0)
            h.update(maxs=maxs, idxu=idxu)
            return h

        def emit_post_k(h):
            """Rank inversion + k gather — emitted right after the tile's own
            trio so the GPSIMD round-trips complete before they are needed."""
            idxu, cklo, ckhi = h["idxu"], h["cklo"], h["ckhi"]
            rankp1 = pool.tile([128, CW], I16, tag="rankp1")
            nc.gpsimd.local_scatter(
                rankp1[:], iota1[:], idxu[:].bitcast(I16),
                channels=128, num_elems=CW, num_idxs=TOPK)
            rankm1 = pool.tile([128, CW], I16, tag="rankm1")
            nc.scalar.activation(rankm1[:], rankp1[:], AF.Identity, bias=negone[:])
            kglo = pool.tile([128, TOPK], U16, tag="kglo")
            kghi = pool.tile([128, TOPK], U16, tag="kghi")
            nc.gpsimd.local_scatter(kglo[:], cklo[:], rankm1[:],
                                    channels=128, num_elems=TOPK, num_idxs=CW)
            nc.gpsimd.local_scatter(kghi[:], ckhi[:], rankm1[:],
                                    channels=128, num_elems=TOPK, num_idxs=CW)
            tkk = pool.tile([128, TOPK], F32, tag="tkk")
            tkku = tkk[:].bitcast(U16)
            nc.scalar.activation(tkku[:, 0 : 2 * TOPK : 2], kglo[:], AF.Copy)
            nc.scalar.activation(tkku[:, 1 : 2 * TOPK : 2], kghi[:], AF.Copy)
            h.update(tkk=tkk)
            return h

        def emit_post_soft(h):
            """Softmax pieces, transposes, final conv (deferred one tile)."""
            p0, rs, maxs, tkk = h["p0"], h["rs"], h["maxs"], h["tkk"]
            negm = tiny("negm")
            nc.scalar.activation(negm[:], maxs[:, 0:1], AF.Identity, scale=-1.0)
            junk2 = pool.tile([128, M], F32, tag="r", name="junk2")  # scratch
            zsum = tiny("zsum")
            nc.scalar.activation(junk2[:], rs[:], AF.Exp, bias=negm[:],
                                 accum_out=zsum[:])
            rz = tiny("rz")
            nc.vector.reciprocal(rz[:], zsum[:])
            esort = pool.tile([128, TOPK], F32, tag="esort")
            nc.scalar.activation(esort[:], maxs[:], AF.Exp, bias=negm[:])
            nc.scalar.activation(esort[:], esort[:], AF.Copy, bias=0.0, scale=rz[:])

            # y^T via PE transpose
            yt1 = pool.tile([128, 128], F32, tag="yt1")
            yt2 = pool.tile([64, 128], F32, tag="yt2")
            yt3 = pool.tile([128, 128], F32, tag="yt3")
            yt4 = pool.tile([64, 128], F32, tag="yt4")
            for src, dst, width in ((esort[:, 0:128], yt1, 128),
                                    (esort[:, 128:192], yt2, 64),
                                    (tkk[:, 0:128], yt3, 128),
                                    (tkk[:, 128:192], yt4, 64)):
                tps = psum.tile([width, 128], F32, name="tps", tag="tps")
                nc.tensor.transpose(tps[:], src, ident[:])
                nc.scalar.activation(dst[:], tps[:], AF.Identity)

            # final 1x1 conv
            outp = psum.tile([OC, 128], F32, tag="outp")
            nc.tensor.matmul(outp[:], wf1[:], yt1[:], start=True, stop=False)
            nc.tensor.matmul(outp[:], wf2[:], yt2[:], start=False, stop=False)
            nc.tensor.matmul(outp[:], wf3[:], yt3[:], start=False, stop=False)
            nc.tensor.matmul(outp[:], wf4[:], yt4[:], start=False, stop=True)
            outsb = pool.tile([OC, 128], F32, tag="outsb")
            nc.scalar.activation(outsb[:], outp[:], AF.Identity, bias=bconv[:])
            nc.sync.dma_start(out_d[:, p0 : p0 + 128], outsb[:])

        # software pipeline:
        #   fa(0); prep(0)
        #   for i: fa(i+1); trio(i){mid: prep(i+1)}; post_k(i); post_soft(i-1)
        #   post_soft(ntiles-1)
        hs = [None] * ntiles
        hs[0] = emit_prep(emit_fa(0))
        for i in range(ntiles):
            if i + 1 < ntiles:
                hs[i + 1] = emit_fa(i + 1)
                mid = (lambda j=i + 1: emit_prep(hs[j]))
            else:
                mid = None
            emit_trio(hs[i], mid=mid)
            emit_post_k(hs[i])
            if i >= 1:
                emit_post_soft(hs[i - 1])
        emit_post_soft(hs[ntiles - 1])

    nc.compile()
    return nc


def host_inputs(x, w_r, w_k, w_conv, b_conv):
    """Build the per-core in_maps (host side: only slicing/layout, no math)."""
    import ml_dtypes
    wr = w_r[:, 0]  # (576, 3, 3)
    wk = w_k[:, 0]
    g = np.arange(M) // 9  # group (input channel) of each output channel

    def dual(wv, dy):  # (128, 576): rows 0:64 tap (dy,-1), rows 64:128 tap (dy,0)
        m = np.zeros((128, M), np.float32)
        m[g, np.arange(M)] = wv[:, dy, 0]
        m[64 + g, np.arange(M)] = wv[:, dy, 1]
        return m

    def single(wv, dy):  # (64, 576): tap (dy,+1)
        m = np.zeros((64, M), np.float32)
        m[g, np.arange(M)] = wv[:, dy, 2]
        return m

    wdr = np.stack([dual(wr, d) for d in range(3)])
    wsr = np.stack([single(wr, d) for d in range(3)])
    wdk = np.stack([dual(wk, d) for d in range(3)]).astype(ml_dtypes.bfloat16)
    wsk = np.stack([single(wk, d) for d in range(3)]).astype(ml_dtypes.bfloat16)
    wfin = np.ascontiguousarray(w_conv[:, :, 0, 0].T.astype(np.float32))  # (384, 128)
    bc = np.ascontiguousarray(b_conv.astype(np.float32).reshape(OC, 1))
    ident = np.eye(128, dtype=np.float32)
    iota1 = np.tile(np.arange(1, TOPK + 1, dtype=np.int16), (128, 1))
    negone = np.full((128, 1), -1.0, np.float32)
    cb = np.tile(np.array([1.5 * Y0, 1.5, M - 2 * CT, M - 2 * CT2], np.float32),
                 (128, 1))
    consts = dict(wdr=wdr, wsr=wsr, wdk=wdk, wsk=wsk, wfin=wfin, bconv=bc,
                  ident=ident, iota1=iota1, negone=negone, cb=cb)
    return [dict(x3=np.ascontiguousarray(x[b].astype(np.float32)),
                 x3b=np.ascontiguousarray(x[b].astype(ml_dtypes.bfloat16)),
                 **consts)
            for b in range(NB)]


def kernel(x, w_r, w_k, w_conv, b_conv):
    if "nc" not in _CACHE:
        _CACHE["nc"] = build()
    nc = _CACHE["nc"]
    in_maps = host_inputs(np.asarray(x), np.asarray(w_r), np.asarray(w_k),
                          np.asarray(w_conv), np.asarray(b_conv))
    res = run_bass_kernel_spmd(nc, in_maps, list(range(NB)))
    out = np.stack([res.results[b]["out"] for b in range(NB)], axis=0)
    return out.reshape(NB, OC, H, W).astype(np.float32)


# revision 46
# speedup vs baseline: 1.1558x; 1.0032x over previous
"""Trainium2 Bass kernel for nn_DefConv_49005576848085 (topk_masking).

Computes, per batch image (data-parallel over 8 NeuronCores):
  r = dwconv3x3(x, w_r); k = dwconv3x3(x, w_k)            # (576, 96, 96)
  per pixel: softmax over 576 channels of r, top-192 (sorted desc, stable),
  gather k at the top-192 indices, y = [top_r_softmax ; top_k] (384),
  out = w_conv @ y + b_conv                               # (128, 96, 96)

Device pipeline per 128-pixel tile (threshold-compaction top-k):
  PE   : r-conv fp32 + k-conv bf16 as tap-window matmuls -> PSUM
  ACT  : PSUM drains (+ mean/sq accumulators), per-pixel Gaussian-quantile
         threshold theta (Newton rsqrt + 2 exact-count refinements via Sign
         accumulate), shift r by -theta, 16-bit pack/unpack copies
  DVE  : prefix-scan compaction indices, then iterative exact top-8
         extraction x24 over the CW=232-wide compacted positive array
         (max8 / find_index8 / match_replace-to-zero)
  GPSIMD: local_scatter compaction of (r-theta, k) 16-bit halves,
          rank-inversion + 16bit-pair scatter = k-gather
  PE   : transpose sorted arrays, 1x1 conv matmuls (+bias via ACT) -> out

Software pipeline: tile i+1's compaction prep (DVE scan + GPSIMD scatters +
ACT assemble) is injected into the middle of tile i's extraction loop so the
compact array is always ready when the next extraction starts.
"""
import numpy as np
from contextlib import ExitStack

import concourse.bass as bass
import concourse.tile as tile
import concourse.mybir as mybir
from concourse import bacc, library_config
from concourse.bass_utils import run_bass_kernel_spmd

C = 64
M = 576          # C*3*3 conv output channels
OC = 128
TOPK = 192
H = W = 96
NPIX = H * W     # 9216
NB = 8           # batch == cores
NIT = TOPK // 8  # 24 extraction iterations
CW = 232         # compact array width (validated count range [200, 227])
NSPLIT = 12      # trio iteration at which next tile's prep is injected

# threshold constants: target count Ct=224 => z0 = ppf(1-224/576),
# K0 = 1/(576*pdf(z0)); y0 = Newton rsqrt seed for var in [0.115, 0.551];
# second refinement targets Ct2=216 with damping 0.7
CT = 224.0
CT2 = 216.0
DAMP2 = 0.7
Z0 = 0.282216147
K0 = 0.004528583
Y0 = 1.8491254

F32 = mybir.dt.float32
F16 = mybir.dt.float16
BF16 = mybir.dt.bfloat16
I16 = mybir.dt.int16
U16 = mybir.dt.uint16
AF = mybir.ActivationFunctionType
OP = mybir.AluOpType

_CACHE = {}


def build(ntiles=NPIX // 128):
    nc = bacc.Bacc("TRN2", target_bir_lowering=False, debug=False, num_devices=NB)

    x3 = nc.dram_tensor("x3", [C, H, W], F32, kind="ExternalInput").ap()
    x3b = nc.dram_tensor("x3b", [C, H, W], BF16, kind="ExternalInput").ap()
    wdr_d = nc.dram_tensor("wdr", [3, 128, M], F32, kind="ExternalInput").ap()
    wsr_d = nc.dram_tensor("wsr", [3, 64, M], F32, kind="ExternalInput").ap()
    wdk_d = nc.dram_tensor("wdk", [3, 128, M], BF16, kind="ExternalInput").ap()
    wsk_d = nc.dram_tensor("wsk", [3, 64, M], BF16, kind="ExternalInput").ap()
    wfin_d = nc.dram_tensor("wfin", [2 * TOPK, OC], F32, kind="ExternalInput").ap()
    bconv_d = nc.dram_tensor("bconv", [OC, 1], F32, kind="ExternalInput").ap()
    ident_d = nc.dram_tensor("ident", [128, 128], F32, kind="ExternalInput").ap()
    iota1_d = nc.dram_tensor("iota1", [128, TOPK], I16, kind="ExternalInput").ap()
    negone_d = nc.dram_tensor("negone", [128, 1], F32, kind="ExternalInput").ap()
    cb_d = nc.dram_tensor("cb", [128, 4], F32, kind="ExternalInput").ap()
    out_d = nc.dram_tensor("out", [OC, NPIX], F32, kind="ExternalOutput").ap()

    with tile.TileContext(nc) as tc, ExitStack() as ctx:
        nc.gpsimd.load_library(library_config.local_scatter)

        cpool = ctx.enter_context(tc.tile_pool(name="const", bufs=1))
        # x tap-shift planes:
        #  XP partitions 0:64   = X_{-1}[c, q] = x[c, row(q), col(q)-1]  (0 at col 0)
        #  XP partitions 64:128 = X_0  [c, q] = x[c, q]
        #  XQ partitions 0:64   = X_{+1}[c, q] = x[c, row(q), col(q)+1]  (0 at col 95)
        # stored with one zero row before and after (98 rows of 96).
        XP = cpool.tile([128, H + 2, W], F32)
        XQ = cpool.tile([64, H + 2, W], F32)
        XPb = cpool.tile([128, H + 2, W], BF16)
        XQb = cpool.tile([64, H + 2, W], BF16)
        XPf = XP[:].rearrange("p a b -> p (a b)")
        XQf = XQ[:].rearrange("p a b -> p (a b)")
        XPbf = XPb[:].rearrange("p a b -> p (a b)")
        XQbf = XQb[:].rearrange("p a b -> p (a b)")
        for P_, Q_, src in ((XP, XQ, x3), (XPb, XQb, x3b)):
            nc.vector.memset(P_[:, 0, :], 0.0)
            nc.vector.memset(P_[:, H + 1, :], 0.0)
            nc.vector.memset(P_[0:64, 1 : H + 1, 0:1], 0.0)
            nc.vector.memset(Q_[:, 0, :], 0.0)
            nc.vector.memset(Q_[:, H + 1, :], 0.0)
            nc.vector.memset(Q_[0:64, 1 : H + 1, W - 1 : W], 0.0)
            nc.sync.dma_start(P_[64:128, 1 : H + 1, :], src[:, :, :])
            nc.sync.dma_start(P_[0:64, 1 : H + 1, 1:W], src[:, :, 0 : W - 1])
            nc.sync.dma_start(Q_[0:64, 1 : H + 1, 0 : W - 1], src[:, :, 1:W])

        wdr = [cpool.tile([128, M], F32, name=f"wdr{d}", tag=f"wdr{d}") for d in range(3)]
        wsr = [cpool.tile([64, M], F32, name=f"wsr{d}", tag=f"wsr{d}") for d in range(3)]
        wdk = [cpool.tile([128, M], BF16, name=f"wdk{d}", tag=f"wdk{d}") for d in range(3)]
        wsk = [cpool.tile([64, M], BF16, name=f"wsk{d}", tag=f"wsk{d}") for d in range(3)]
        for d in range(3):
            nc.sync.dma_start(wdr[d][:], wdr_d[d])
            nc.sync.dma_start(wsr[d][:], wsr_d[d])
            nc.sync.dma_start(wdk[d][:], wdk_d[d])
            nc.sync.dma_start(wsk[d][:], wsk_d[d])
        wf1 = cpool.tile([128, OC], F32)
        wf2 = cpool.tile([64, OC], F32)
        wf3 = cpool.tile([128, OC], F32)
        wf4 = cpool.tile([64, OC], F32)
        nc.sync.dma_start(wf1[:], wfin_d[0:128])
        nc.sync.dma_start(wf2[:], wfin_d[128:192])
        nc.sync.dma_start(wf3[:], wfin_d[192:320])
        nc.sync.dma_start(wf4[:], wfin_d[320:384])
        ident = cpool.tile([128, 128], F32)
        nc.sync.dma_start(ident[:], ident_d[:])
        iota1 = cpool.tile([128, TOPK], I16)
        nc.sync.dma_start(iota1[:], iota1_d[:])
        bconv = cpool.tile([OC, 1], F32)
        nc.sync.dma_start(bconv[:], bconv_d[:])
        negone = cpool.tile([128, 1], F32)
        nc.sync.dma_start(negone[:], negone_d[:])
        cb = cpool.tile([128, 4], F32)  # cols: [1.5*Y0, 1.5, 576-2*Ct, 576-2*Ct2]
        nc.sync.dma_start(cb[:], cb_d[:])
        zeros = cpool.tile([128, M], F32)
        nc.vector.memset(zeros[:], 0.0)

        pool = ctx.enter_context(tc.tile_pool(name="work", bufs=2))
        pool3 = ctx.enter_context(tc.tile_pool(name="work3", bufs=3))
        psum = ctx.enter_context(tc.tile_pool(name="psum", bufs=1, space="PSUM"))

        def tiny(tag):
            return pool.tile([128, 1], F32, tag=tag, name=tag)

        def emit_fa(it):
            """Convs, drains, theta estimation, shifted r for tile `it`.
            No DVE work (so it can be emitted ahead of the previous trio)."""
            p0 = 128 * it
            pr1 = psum.tile([128, 288], F32, tag="pr1")
            pr2 = psum.tile([128, 288], F32, tag="pr2")
            pk1 = psum.tile([128, 288], F32, tag="pk1")
            pk2 = psum.tile([128, 288], F32, tag="pk2")
            for d in range(3):  # dy = d - 1; taps (dy,-1),(dy,0) dual; (dy,+1) single
                w0 = 96 * d + p0
                lhd = XPf[:, w0 : w0 + 128]
                lhs = XQf[0:64, w0 : w0 + 128]
                lhdb = XPbf[:, w0 : w0 + 128]
                lhsb = XQbf[0:64, w0 : w0 + 128]
                st = d == 0
                sp = d == 2
                nc.tensor.matmul(pr1[:], lhd, wdr[d][:, 0:288], start=st, stop=False)
                nc.tensor.matmul(pr2[:], lhd, wdr[d][:, 288:M], start=st, stop=False)
                nc.tensor.matmul(pk1[:], lhdb, wdk[d][:, 0:288], start=st, stop=False)
                nc.tensor.matmul(pk2[:], lhdb, wdk[d][:, 288:M], start=st, stop=False)
                nc.tensor.matmul(pr1[:], lhs, wsr[d][:, 0:288], start=False, stop=sp)
                nc.tensor.matmul(pr2[:], lhs, wsr[d][:, 288:M], start=False, stop=sp)
                nc.tensor.matmul(pk1[:], lhsb, wsk[d][:, 0:288], start=False, stop=sp)
                nc.tensor.matmul(pk2[:], lhsb, wsk[d][:, 288:M], start=False, stop=sp)

            # drains + stats; rs doubles as scratch for Square/Sign passes
            r = pool.tile([128, M], F32, tag="r")
            kv = pool.tile([128, M], F32, tag="kv")
            rs = pool3.tile([128, M], F32, tag="rs")
            s1 = tiny("s1")
            s2 = tiny("s2")
            q1 = tiny("q1")
            q2 = tiny("q2")
            nc.scalar.activation(r[:, 0:288], pr1[:], AF.Identity, accum_out=s1[:])
            nc.scalar.activation(r[:, 288:M], pr2[:], AF.Identity, accum_out=s2[:])
            nc.scalar.activation(kv[:, 0:288], pk1[:], AF.Identity)
            nc.scalar.activation(kv[:, 288:M], pk2[:], AF.Identity)
            nc.scalar.activation(rs[:, 0:288], pr1[:], AF.Square, accum_out=q1[:])
            nc.scalar.activation(rs[:, 288:M], pr2[:], AF.Square, accum_out=q2[:])

            # theta estimate (tiny ACT chain; out = func(in*scale + bias))
            S = tiny("S")
            q = tiny("q")
            nc.scalar.activation(S[:], s1[:], AF.Identity, bias=s2[:])
            nc.scalar.activation(q[:], q1[:], AF.Identity, bias=q2[:])
            negmu = tiny("negmu")
            msq = tiny("msq")
            mu2 = tiny("mu2")
            var = tiny("var")
            nc.scalar.activation(negmu[:], S[:], AF.Identity, scale=-1.0 / M)
            nc.scalar.activation(msq[:], q[:], AF.Identity, scale=1.0 / M)
            nc.scalar.activation(mu2[:], negmu[:], AF.Square)
            nc.scalar.activation(var[:], mu2[:], AF.Identity, scale=-1.0, bias=msq[:])
            # sg ~= sqrt(var) via 1 Newton rsqrt step from seed Y0 (the two
            # exact-count refinements absorb the remaining ~10% sg error)
            y1 = tiny("y1")
            sg = tiny("sg")
            nc.scalar.activation(y1[:], var[:], AF.Identity,
                                 scale=-0.5 * Y0 ** 3, bias=cb[:, 0:1])
            nc.scalar.activation(sg[:], var[:], AF.Identity, scale=y1[:])
            negth0 = tiny("negth0")
            nc.scalar.activation(negth0[:], sg[:], AF.Identity, scale=-Z0,
                                 bias=negmu[:])
            # refinement 1: exact count at theta0 via Sign accumulate
            # (ssum = 2*count - 576):
            #   negth1 = negth0 + (ssum + (576-2*Ct)) * (-K0/2*sg)
            ssum = tiny("ssum")
            nc.scalar.activation(rs[:], r[:], AF.Sign, bias=negth0[:],
                                 accum_out=ssum[:])
            uK2 = tiny("uK2")
            sh = tiny("sh")
            d2 = tiny("d2")
            negth1 = tiny("negth1")
            nc.scalar.activation(uK2[:], sg[:], AF.Identity, scale=-K0 / 2.0)
            nc.scalar.activation(sh[:], ssum[:], AF.Identity, bias=cb[:, 2:3])
            nc.scalar.activation(d2[:], sh[:], AF.Identity, scale=uK2[:])
            nc.scalar.activation(negth1[:], d2[:], AF.Identity, bias=negth0[:])
            # refinement 2 (damped) targeting Ct2
            ssum2 = tiny("ssum2")
            nc.scalar.activation(rs[:], r[:], AF.Sign, bias=negth1[:],
                                 accum_out=ssum2[:])
            uK3 = tiny("uK3")
            sh2 = tiny("sh2")
            d3 = tiny("d3")
            negth2 = tiny("negth2")
            nc.scalar.activation(uK3[:], sg[:], AF.Identity,
                                 scale=-K0 * DAMP2 / 2.0)
            nc.scalar.activation(sh2[:], ssum2[:], AF.Identity, bias=cb[:, 3:4])
            nc.scalar.activation(d3[:], sh2[:], AF.Identity, scale=uK3[:])
            nc.scalar.activation(negth2[:], d3[:], AF.Identity, bias=negth1[:])
            # shifted r (>= 0 exactly on the kept elements)
            nc.scalar.activation(rs[:], r[:], AF.Identity, bias=negth2[:])
            return dict(p0=p0, r=r, kv=kv, rs=rs)

        def emit_prep(h):
            """Compaction for tile of `h`: DVE scan + scatters + assemble.
            Returns handles for the trio + post."""
            rs, kv = h["rs"], h["kv"]
            ind = pool.tile([128, M], BF16, tag="ind")
            nc.vector.tensor_scalar(ind[:], rs[:], 0.0, None, OP.is_ge)
            pc = pool.tile([128, M], F32, tag="r")  # r is dead; reuse as pc
            nc.vector.tensor_tensor_scan(pc[:], ind[:], zeros[:], 0.0, OP.add, OP.add)
            siF = pool.tile([128, M], F16, tag="siF")
            nc.vector.scalar_tensor_tensor(siF[:], rs[:], 0.0, pc[:], OP.is_ge, OP.mult)
            si16 = pool.tile([128, M], I16, tag="si16")
            nc.scalar.activation(si16[:], siF[:], AF.Identity, bias=negone[:])

            rslo = pool.tile([128, M], U16, tag="rslo")
            rshi = pool.tile([128, M], U16, tag="rshi")
            rsu = rs[:].bitcast(U16)
            nc.scalar.activation(rslo[:], rsu[:, 0 : 2 * M : 2], AF.Copy)
            nc.scalar.activation(rshi[:], rsu[:, 1 : 2 * M : 2], AF.Copy)
            aclo = pool.tile([128, CW], U16, tag="aclo")
            achi = pool.tile([128, CW], U16, tag="achi")
            nc.gpsimd.local_scatter(aclo[:], rslo[:], si16[:],
                                    channels=128, num_elems=CW, num_idxs=M)
            nc.gpsimd.local_scatter(achi[:], rshi[:], si16[:],
                                    channels=128, num_elems=CW, num_idxs=M)
            ac = pool.tile([128, CW], F32, tag="ac")
            acu = ac[:].bitcast(U16)
            nc.scalar.activation(acu[:, 0 : 2 * CW : 2], aclo[:], AF.Copy)
            nc.scalar.activation(acu[:, 1 : 2 * CW : 2], achi[:], AF.Copy)

            # k compaction (consumed only by post; scatters overlap the trio)
            klo = pool.tile([128, M], U16, tag="klo")
            khi = pool.tile([128, M], U16, tag="khi")
            kvu = kv[:].bitcast(U16)
            nc.scalar.activation(klo[:], kvu[:, 0 : 2 * M : 2], AF.Copy)
            nc.scalar.activation(khi[:], kvu[:, 1 : 2 * M : 2], AF.Copy)
            cklo = pool.tile([128, CW], U16, tag="cklo")
            ckhi = pool.tile([128, CW], U16, tag="ckhi")
            nc.gpsimd.local_scatter(cklo[:], klo[:], si16[:],
                                    channels=128, num_elems=CW, num_idxs=M)
            nc.gpsimd.local_scatter(ckhi[:], khi[:], si16[:],
                                    channels=128, num_elems=CW, num_idxs=M)
            h.update(ac=ac, cklo=cklo, ckhi=ckhi)
            return h

        def emit_trio(h, mid=None):
            """Top-192 extraction for tile of `h`; `mid` emits the next
            tile's prep after iteration NSPLIT so its scatters overlap."""
            ac = h["ac"]
            maxs = pool3.tile([128, TOPK], F32, tag="maxs")
            idxu = pool.tile([128, TOPK], U16, tag="idxu")
            for t in range(NIT):
                if t == NSPLIT and mid is not None:
                    mid()
                m8 = maxs[:, 8 * t : 8 * t + 8]
                nc.vector.max(out=m8, in_=ac[:])
                nc.vector.max_index(out=idxu[:, 8 * t : 8 * t + 8], in_max=m8,
                                    in_values=ac[:])
                if t < NIT - 1:
                    # zero out the extracted 8 (all values >= 0 in the shifted
                    # domain, so 0 acts as -inf)
                    nc.vector.match_replace(out=ac[:], in_to_replace=m8,
                                            in_values=ac[:], imm_value=0.0)
            h.update(maxs=maxs, idxu=idxu)
            return h

        def emit_post_k(h):
            """Rank inversion + k gather — emitted right after the tile's own
            trio so the GPSIMD round-trips complete before they are needed."""
            idxu, cklo, ckhi = h["idxu"], h["cklo"], h["ckhi"]
            rankp1 = pool.tile([128, CW], I16, tag="rankp1")
            nc.gpsimd.local_scatter(
                rankp1[:], iota1[:], idxu[:].bitcast(I16),
                channels=128, num_elems=CW, num_idxs=TOPK)
            rankm1 = pool.tile([128, CW], I16, tag="rankm1")
            nc.scalar.activation(rankm1[:], rankp1[:], AF.Identity, bias=negone[:])
            kglo = pool.tile([128, TOPK], U16, tag="kglo")
            kghi = pool.tile([128, TOPK], U16, tag="kghi")
            nc.gpsimd.local_scatter(kglo[:], cklo[:], rankm1[:],
                                    channels=128, num_elems=TOPK, num_idxs=CW)
            nc.gpsimd.local_scatter(kghi[:], ckhi[:], rankm1[:],
                                    channels=128, num_elems=TOPK, num_idxs=CW)
            tkk = pool.tile([128, TOPK], F32, tag="tkk")
            tkku = tkk[:].bitcast(U16)
            nc.scalar.activation(tkku[:, 0 : 2 * TOPK : 2], kglo[:], AF.Copy)
            nc.scalar.activation(tkku[:, 1 : 2 * TOPK : 2], kghi[:], AF.Copy)
            h.update(tkk=tkk)
            return h

        def emit_post_soft(h):
            """Softmax pieces, transposes, final conv (deferred one tile)."""
            p0, rs, maxs, tkk = h["p0"], h["rs"], h["maxs"], h["tkk"]
            negm = tiny("negm")
            nc.scalar.activation(negm[:], maxs[:, 0:1], AF.Identity, scale=-1.0)
            junk2 = pool.tile([128, M], F32, tag="r", name="junk2")  # scratch
            zsum = tiny("zsum")
            nc.scalar.activation(junk2[:], rs[:], AF.Exp, bias=negm[:],
                                 accum_out=zsum[:])
            rz = tiny("rz")
            nc.vector.reciprocal(rz[:], zsum[:])
            esort = pool.tile([128, TOPK], F32, tag="esort")
            nc.scalar.activation(esort[:], maxs[:], AF.Exp, bias=negm[:])
            nc.scalar.activation(esort[:], esort[:], AF.Copy, bias=0.0, scale=rz[:])

            # y^T via PE transpose
            yt1 = pool.tile([128, 128], F32, tag="yt1")
            yt2 = pool.tile([64, 128], F32, tag="yt2")
            yt3 = pool.tile([128, 128], F32, tag="yt3")
            yt4 = pool.tile([64, 128], F32, tag="yt4")
            for src, dst, width in ((esort[:, 0:128], yt1, 128),
                                    (esort[:, 128:192], yt2, 64),
                                    (tkk[:, 0:128], yt3, 128),
                                    (tkk[:, 128:192], yt4, 64)):
                tps = psum.tile([width, 128], F32, name="tps", tag="tps")
                nc.tensor.transpose(tps[:], src, ident[:])
                nc.scalar.activation(dst[:], tps[:], AF.Identity)

            # final 1x1 conv
            outp = psum.tile([OC, 128], F32, tag="outp")
            nc.tensor.matmul(outp[:], wf1[:], yt1[:], start=True, stop=False)
            nc.tensor.matmul(outp[:], wf2[:], yt2[:], start=False, stop=False)
            nc.tensor.matmul(outp[:], wf3[:], yt3[:], start=False, stop=False)
            nc.tensor.matmul(outp[:], wf4[:], yt4[:], start=False, stop=True)
            outsb = pool.tile([OC, 128], F32, tag="outsb")
            nc.scalar.activation(outsb[:], outp[:], AF.Identity, bias=bconv[:])
            nc.sync.dma_start(out_d[:, p0 : p0 + 128], outsb[:])

        # software pipeline:
        #   fa(0); prep(0)
        #   for i: fa(i+1); trio(i){mid: prep(i+1)}; post_k(i); post_soft(i-1)
        #   post_soft(ntiles-1)
        hs = [None] * ntiles
        hs[0] = emit_prep(emit_fa(0))
        for i in range(ntiles):
            if i + 1 < ntiles:
                hs[i + 1] = emit_fa(i + 1)
                mid = (lambda j=i + 1: emit_prep(hs[j]))
            else:
                mid = None
            emit_trio(hs[i], mid=mid)
            emit_post_k(hs[i])
            if i >= 1:
                emit_post_soft(hs[i - 1])
        emit_post_soft(hs[ntiles - 1])

    nc.compile()
    return nc


def host_inputs(x, w_r, w_k, w_conv, b_conv):
    """Build the per-core in_maps (host side: only slicing/layout, no math)."""
    import ml_dtypes
    wr = w_r[:, 0]  # (576, 3, 3)
    wk = w_k[:, 0]
    g = np.arange(M) // 9  # group (input channel) of each output channel

    def dual(wv, dy):  # (128, 576): rows 0:64 tap (dy,-1), rows 64:128 tap (dy,0)
        m = np.zeros((128, M), np.float32)
        m[g, np.arange(M)] = wv[:, dy, 0]
        m[64 + g, np.arange(M)] = wv[:, dy, 1]
        return m

    def single(wv, dy):  # (64, 576): tap (dy,+1)
        m = np.zeros((64, M), np.float32)
        m[g, np.arange(M)] = wv[:, dy, 2]
        return m

    wdr = np.stack([dual(wr, d) for d in range(3)])
    wsr = np.stack([single(wr, d) for d in range(3)])
    wdk = np.stack([dual(wk, d) for d in range(3)]).astype(ml_dtypes.bfloat16)
    wsk = np.stack([single(wk, d) for d in range(3)]).astype(ml_dtypes.bfloat16)
    wfin = np.ascontiguousarray(w_conv[:, :, 0, 0].T.astype(np.float32))  # (384, 128)
    bc = np.ascontiguousarray(b_conv.astype(np.float32).reshape(OC, 1))
    ident = np.eye(128, dtype=np.float32)
    iota1 = np.tile(np.arange(1, TOPK + 1, dtype=np.int16), (128, 1))
    negone = np.full((128, 1), -1.0, np.float32)
    cb = np.tile(np.array([1.5 * Y0, 1.5, M - 2 * CT, M - 2 * CT2], np.float32),
                 (128, 1))
    consts = dict(wdr=wdr, wsr=wsr, wdk=wdk, wsk=wsk, wfin=wfin, bconv=bc,
                  ident=ident, iota1=iota1, negone=negone, cb=cb)
    return [dict(x3=np.ascontiguousarray(x[b].astype(np.float32)),
                 x3b=np.ascontiguousarray(x[b].astype(ml_dtypes.bfloat16)),
                 **consts)
            for b in range(NB)]


def kernel(x, w_r, w_k, w_conv, b_conv):
    if "nc" not in _CACHE:
        _CACHE["nc"] = build()
    nc = _CACHE["nc"]
    in_maps = host_inputs(np.asarray(x), np.asarray(w_r), np.asarray(w_k),
                          np.asarray(w_conv), np.asarray(b_conv))
    res = run_bass_kernel_spmd(nc, in_maps, list(range(NB)))
    out = np.stack([res.results[b]["out"] for b in range(NB)], axis=0)
    return out.reshape(NB, OC, H, W).astype(np.float32)
